# revision 17
# baseline (speedup 1.0000x reference)
"""Trainium2 Bass kernel for nn_CausalSelfAttention_55207509622867.

Sharding: 8 cores = 4 batches x 2 head-groups (8 heads each).
Per-core SPMD program (same IR, different data):
  - QKV projection (bf16 matmuls, fp32 PSUM accumulate)
  - RMS-norm + rotary on Q,K in natural [t,d] layout (DVE)
  - PE-transpose Q,K to [d,t] layout for attention
  - windowed-causal attention in S^T=[k,q] layout; mask via identity-matmul
    bias injection into PSUM (-400 out-of-band); exp on ScalarE;
    softmax denominator via an all-ones column appended to V (row 64 of O^T)
  - normalize O^T with PE-broadcast of 1/den, then c_proj matmuls
Host: pre-transposes/casts weights and x, blends value output, sums the
two head-group partial y's, adds bias, reassembles full outputs.
"""

import numpy as np
import ml_dtypes

import bass_rust
import concourse.bass as bass
import concourse.tile as tile
import concourse.mybir as mybir
from concourse.bass_utils import run_bass_kernel_spmd
from concourse.vector_clock import ScopedClock

# ----------------------------------------------------------------------------
# Tile exit-drain patch: this walrus build rejects sem waits on InstDrain
# ("Too many sync wait commands"). Put the global-clock waits on preceding
# sync-engine nops (1 wait each), then emit a bare drain.
# ----------------------------------------------------------------------------
def _patched_drain_and_barrier(self, tick_clock, wait_clock):
    nc = self.nc
    probe = nc.sync.nop(nofuse=True)
    wait_clock.add_sem_waits(probe.ins, ScopedClock({None: tick_clock.global_clock}))
    si = probe.ins.sync_info
    waits = [] if si is None else list(si.on_wait)
    if len(waits) > 1:
        probe.ins.sync_info = bass_rust.SyncInfo(
            on_wait=waits[:1], on_update=[] if si is None else list(si.on_update)
        )
        for w in waits[1:]:
            nop = nc.sync.nop(nofuse=True)
            nop.ins.sync_info = bass_rust.SyncInfo(on_wait=[w], on_update=[])
    nc.sync.drain()
    nc.all_engine_barrier()
    assert self.sems is not None
    popped = nc._tile_sem_poison_stack.pop()
    assert popped is self._sem_poison
    nc.clear_and_free_semaphores(list(self.sems.allocated().values()))
    nc.all_engine_barrier()


tile.TileContext._drain_and_barrier = _patched_drain_and_barrier

_MAXW = 1  # max sync waits this walrus accepts per instruction


def _split_excess_waits(nc):
    """Move waits beyond _MAXW onto same-engine nops inserted just before."""
    k = 0
    for bb in nc.m.functions[0].blocks:
        out = []
        for inst in bb.instructions:
            si = inst.sync_info
            waits = [] if si is None else list(si.on_wait)
            if len(waits) > _MAXW:
                extra = waits[:-_MAXW]
                inst.sync_info = bass_rust.SyncInfo(
                    on_wait=waits[-_MAXW:],
                    on_update=list(si.on_update),
                )
                for i in range(0, len(extra), _MAXW):
                    nop = mybir.InstNoOp(name=f"I-waitsplit-{k}", ins=[], outs=[])
                    k += 1
                    nop.engine = inst.engine
                    nop.sync_info = bass_rust.SyncInfo(
                        on_wait=extra[i : i + _MAXW], on_update=[]
                    )
                    out.append(nop)
            out.append(inst)
        if k:
            bb.instructions = out
    return k

# ----------------------------------------------------------------------------
# Problem constants (hardcoded per task contract)
# ----------------------------------------------------------------------------
B, T, C, H, HD = 4, 2048, 1024, 16, 64
W = 512                      # attention window
HPC = 8                      # heads per core
D = HPC * HD                 # 512 head-dims per core
TQ = 256                     # query block
NQB = T // TQ                # 8 query blocks
KT = 128                     # key tile
ROPE_BASE = 1024.0
EPS = 1e-6
NEG = -400.0                 # mask bias (exp(0.125*(-400+s)) == 0)

f32 = mybir.dt.float32
f32r = mybir.dt.float32r
bf16 = mybir.dt.bfloat16
FP = mybir.AluOpType
AF = mybir.ActivationFunctionType

_CACHE = {}


def _ktiles(qb):
    """Valid key-tile start indices (in units of 128) for query block qb."""
    q0 = qb * TQ
    out = []
    for j in range(6):
        k0 = q0 - W + j * KT
        if k0 < 0 or k0 >= T:
            continue
        out.append((j, k0))
    return out


def _mask_r_needed():
    """r = q0-k0 values whose [128,TQ] tile is partially masked."""
    return [-128, 0, 384, 512]


def _mask_tile(r):
    """[128, TQ] fp32 additive-bias tile for relative offset r."""
    p = np.arange(KT)[:, None]
    f = np.arange(TQ)[None, :]
    qk = r + f - p
    ok = (qk >= 0) & (qk < W)
    return np.where(ok, 0.0, NEG).astype(np.float32)


def build_nc():
    nc = bass.Bass()

    # ---- DRAM parameters (per core) ----
    xT_d = nc.declare_dram_parameter("xT", [C, T], bf16, isOutput=False)
    wq_d = nc.declare_dram_parameter("wq_t", [C, D], bf16, isOutput=False)
    wk_d = nc.declare_dram_parameter("wk_t", [C, D], bf16, isOutput=False)
    wv_d = nc.declare_dram_parameter("wv_t", [C, D], bf16, isOutput=False)
    cpw_d = nc.declare_dram_parameter("cpw_t", [D, C], bf16, isOutput=False)
    v1s_d = nc.declare_dram_parameter("v1s", [T, D], f32, isOutput=False)
    cos_d = nc.declare_dram_parameter("cos_r", [T, HPC * 16], bf16, isOutput=False)
    sin_d = nc.declare_dram_parameter("sin_r", [T, HPC * 16], bf16, isOutput=False)
    lam_d = nc.declare_dram_parameter("lam1m", [128, 1], f32, isOutput=False)
    iden_d = nc.declare_dram_parameter("ident_bf", [128, 128], bf16, isOutput=False)
    mask_d = nc.declare_dram_parameter("masks", [4, 128, TQ], bf16, isOutput=False)

    y_d = nc.declare_dram_parameter("y_part", [T, C], f32, isOutput=True)
    val_d = nc.declare_dram_parameter("value_out", [T, D], f32, isOutput=True)
    rden_dram = nc.dram_tensor("rden_scratch", [64, NQB * HPC // 64, TQ], f32)

    NTC = T // 128  # 16 row tiles
    r_list = _mask_r_needed()
    r_slot = {r: i for i, r in enumerate(r_list)}

    with tile.TileContext(nc) as tc:
        from contextlib import ExitStack

        with ExitStack() as root:
            # ------------- persistent tensors -------------
            consts = root.enter_context(tc.tile_pool(name="consts", bufs=1))
            wq_sb = consts.tile([128, 8, D], bf16, name="wq_sb")
            wk_sb = consts.tile([128, 8, D], bf16, name="wk_sb")
            wv_sb = consts.tile([128, 8, D], bf16, name="wv_sb")
            cpw_sb = consts.tile([64, HPC, C], bf16, name="cpw_sb")
            cos_sb = consts.tile([128, NTC, HPC * 16], bf16, name="cos_sb")
            sin_sb = consts.tile([128, NTC, HPC * 16], bf16, name="sin_sb")
            lam_sb = consts.tile([128, 1], f32, name="lam_sb")
            iden_sb = consts.tile([128, 128], bf16, name="iden_sb")
            mask_sb = consts.tile([128, 4, TQ], bf16, name="mask_sb")

            for _ct in range(8):
                nc.sync.dma_start(wq_sb[:, _ct, :], wq_d[_ct * 128 : (_ct + 1) * 128, :])
            for _ct in range(8):
                nc.sync.dma_start(wk_sb[:, _ct, :], wk_d[_ct * 128 : (_ct + 1) * 128, :])
            for _ct in range(8):
                nc.sync.dma_start(wv_sb[:, _ct, :], wv_d[_ct * 128 : (_ct + 1) * 128, :])
            for _h in range(HPC):
                nc.sync.dma_start(cpw_sb[:, _h, :], cpw_d[_h * 64 : (_h + 1) * 64, :])
            nc.sync.dma_start(lam_sb[:], lam_d[:])
            nc.sync.dma_start(iden_sb[:], iden_d[:])
            # masks [4,128,TQ] -> sbuf [128, 4, TQ]
            for i in range(4):
                nc.sync.dma_start(mask_sb[:, i, :], mask_d[i, :, :])
            for tci in range(NTC):
                nc.sync.dma_start(
                    cos_sb[:, tci, :], cos_d[tci * 128 : (tci + 1) * 128, :]
                )
                nc.sync.dma_start(
                    sin_sb[:, tci, :], sin_d[tci * 128 : (tci + 1) * 128, :]
                )

            main = root.enter_context(tc.tile_pool(name="main", bufs=1))
            # V augmented with ones col per head: [t, 8*65]
            v_aug = main.tile([128, NTC, HPC * 65], bf16, name="v_aug")
            # transposed Q,K: head-pair tensors [128, T] bf16
            qT = main.tile([128, 4, T], bf16, name="qT")
            kT = main.tile([128, 4, T], bf16, name="kT")
            # O (unnormalized then normalized in place), per-head, base partition 0
            o_bf = main.tile([64, HPC, T], bf16, name="o_bf")
            den_all = main.tile([64, NQB * HPC // 64, TQ], f32, name="den_all")
            rden_all = main.tile([64, NQB * HPC // 64, TQ], f32, name="rden_all")
            eps_sb = main.tile([128, 1], f32, name="eps_sb")
            nc.gpsimd.memset(eps_sb[:], EPS)

            # ========== Phase 1-3: QKV + rms/rotary + transpose (per tci) ==========
            with tc.tile_pool(name="xload", bufs=8) as xpool, \
                 tc.tile_pool(name="v1load", bufs=3) as v1pool, \
                 tc.tile_pool(name="qkv_ps", bufs=3, space="PSUM") as qkv_ps, \
                 tc.tile_pool(name="valout", bufs=3) as valpool, \
                 tc.tile_pool(name="qknat", bufs=3) as qkpool, \
                 tc.tile_pool(name="ssq", bufs=2) as ssqpool, \
                 tc.tile_pool(name="rstdp", bufs=2) as rstdpool, \
                 tc.tile_pool(name="rot", bufs=4) as rot, \
                 tc.tile_pool(name="tps", bufs=2, space="PSUM") as tps:

                xT_sb = [None] * 8
                for ct in range(8):
                    xt = xpool.tile([128, T], bf16, name=f"xt{ct}", tag="xt")
                    nc.sync.dma_start(xt[:], xT_d[ct * 128 : (ct + 1) * 128, :])
                    xT_sb[ct] = xt

                for tci in range(NTC):
                    ts = bass.ts(tci, 128)
                    qk_nat = qkpool.tile([128, 2 * D], bf16, tag="qkn")
                    # --- Q and K ---
                    for qk, w_sb in ((0, wq_sb), (1, wk_sb)):
                        ps = qkv_ps.tile([128, D], f32, tag="ps")
                        for ct in range(8):
                            nc.tensor.matmul(
                                ps[:],
                                xT_sb[ct][:, ts],
                                w_sb[:, ct, :],
                                start=(ct == 0),
                                stop=(ct == 7),
                            )
                        nc.scalar.copy(qk_nat[:, qk * D : (qk + 1) * D], ps[:])
                    # --- V + blend ---
                    ps = qkv_ps.tile([128, D], f32, tag="ps")
                    for ct in range(8):
                        nc.tensor.matmul(
                            ps[:],
                            xT_sb[ct][:, ts],
                            wv_sb[:, ct, :],
                            start=(ct == 0),
                            stop=(ct == 7),
                        )
                    v1t = v1pool.tile([128, D], f32, tag="v1")
                    nc.sync.dma_start(v1t[:], v1s_d[ts, :])
                    # value = (1-lam)*V + v1s   (fp32, DMA out)
                    vout = valpool.tile([128, D], f32, tag="vo")
                    nc.vector.scalar_tensor_tensor(
                        vout[:], ps[:], lam_sb[:], v1t[:], op0=FP.mult, op1=FP.add
                    )
                    nc.sync.dma_start(val_d[ts, :], vout[:])
                    # V_aug (bf16, gapped by head with ones col)
                    va = v_aug[:, tci, :].rearrange("p (h e) -> p h e", h=HPC)
                    nc.vector.scalar_tensor_tensor(
                        va[:, :, 0:HD],
                        ps[:].rearrange("p (h e) -> p h e", h=HPC),
                        lam_sb[:],
                        v1t[:].rearrange("p (h e) -> p h e", h=HPC),
                        op0=FP.mult,
                        op1=FP.add,
                    )
                    nc.gpsimd.memset(va[:, :, HD : HD + 1], 1.0)

                    # --- rms stats ---
                    sq = ssqpool.tile([128, 2 * D], bf16, tag="sq")
                    nc.vector.tensor_mul(sq[:], qk_nat[:], qk_nat[:])
                    sumsq = rstdpool.tile([128, 2, HPC], f32, tag="ssq")
                    nc.vector.tensor_reduce(
                        sumsq[:],
                        sq[:].rearrange("p (s h d) -> p (s h) d", s=2, h=HPC),
                        axis=mybir.AxisListType.X,
                        op=FP.add,
                    )
                    std_t = rstdpool.tile([128, 2, HPC], f32, tag="std")
                    nc.scalar.activation(
                        std_t[:], sumsq[:], AF.Sqrt, bias=eps_sb[:], scale=1.0 / HD
                    )
                    rstd = rstdpool.tile([128, 2, HPC], f32, tag="rstd")
                    nc.vector.reciprocal(rstd[:], std_t[:])

                    # --- apply rstd + rotary (in place on qk_nat) ---
                    nat = qk_nat[:].rearrange("p (s h d) -> p s h d", s=2, h=HPC)
                    nc.vector.tensor_mul(
                        nat[:],
                        nat[:],
                        rstd[:][:, :, :, None].broadcast_to((128, 2, HPC, HD)),
                    )
                    x1 = nat[:, :, :, 0:16]
                    x2 = nat[:, :, :, 32:48]
                    csb = cos_sb[:, tci, :].rearrange("p (h j) -> p h j", h=HPC)[
                        :, None, :, :
                    ].broadcast_to((128, 2, HPC, 16))
                    ssb = sin_sb[:, tci, :].rearrange("p (h j) -> p h j", h=HPC)[
                        :, None, :, :
                    ].broadcast_to((128, 2, HPC, 16))
                    tb = rot.tile([128, 2, HPC, 16], bf16, tag="tb")
                    td = rot.tile([128, 2, HPC, 16], bf16, tag="td")
                    nc.vector.tensor_mul(tb[:], x1, ssb)   # x1*sin
                    nc.vector.tensor_mul(td[:], x2, ssb)   # x2*sin
                    nc.vector.tensor_mul(x1, x1, csb)      # x1*cos
                    nc.vector.tensor_add(x1, x1, td[:])    # + x2*sin
                    nc.vector.tensor_mul(x2, x2, csb)      # x2*cos
                    nc.vector.tensor_sub(x2, x2, tb[:])    # - x1*sin

                    # --- transpose to [d, t] ---
                    for qk, dst in ((0, qT), (1, kT)):
                        tp = tps.tile([128, 512], bf16, tag="tp")
                        for pair in range(4):
                            nc.tensor.transpose(
                                tp[:, pair * 128 : (pair + 1) * 128],
                                qk_nat[
                                    :, qk * D + pair * 128 : qk * D + (pair + 1) * 128
                                ],
                                iden_sb[:],
                            )
                        nc.vector.tensor_copy(
                            dst[:, :, ts],
                            tp[:].rearrange("p (r c) -> p r c", r=4),
                        )

            # ================= Phase 4: attention =================
            with tc.tile_pool(name="s_ps", bufs=2, space="PSUM") as s_ps, \
                 tc.tile_pool(name="o_ps", bufs=2, space="PSUM") as o_ps, \
                 tc.tile_pool(name="pt_sb", bufs=2) as pt_pool, \
                 tc.tile_pool(name="den_stage", bufs=3) as den_stage:

                for h in range(HPC):
                    pair, half = h // 2, h % 2
                    hslc = slice(half * 64, half * 64 + 64)
                    for qb in range(NQB):
                        q0 = qb * TQ
                        kts = _ktiles(qb)
                        j0 = kts[0][0]
                        sp = s_ps.tile([128, 6 * TQ], f32, tag="sp")
                        for j, k0 in kts:
                            r = q0 - k0
                            slc = slice(j * TQ, (j + 1) * TQ)
                            masked = r in r_slot
                            if masked:
                                nc.tensor.matmul(
                                    sp[:, slc],
                                    iden_sb[:],
                                    mask_sb[:, r_slot[r], :],
                                    start=True,
                                    stop=False,
                                )
                            nc.tensor.matmul(
                                sp[:, slc],
                                kT[hslc, pair, k0 : k0 + 128],
                                qT[hslc, pair, q0 : q0 + TQ],
                                start=not masked,
                                stop=True,
                            )
                        # exp over the valid tail
                        pt = pt_pool.tile([128, 6 * TQ], bf16, tag="pt")
                        nc.scalar.activation(
                            pt[:, j0 * TQ :], sp[:, j0 * TQ :], AF.Exp,
                            scale=1.0 / np.sqrt(HD),
                        )
                        # AV: O^T[65, TQ] accumulate over key tiles
                        op = o_ps.tile([65, TQ], f32, tag="op")
                        for idx, (j, k0) in enumerate(kts):
                            nc.tensor.matmul(
                                op[:],
                                v_aug[:, k0 // 128, h * 65 : (h + 1) * 65],
                                pt[:, j * TQ : (j + 1) * TQ],
                                start=(idx == 0),
                                stop=(idx == len(kts) - 1),
                            )
                        nc.vector.tensor_copy(
                            o_bf[:, h, q0 : q0 + TQ], op[0:64, :]
                        )
                        i = h * NQB + qb
                        dstg = den_stage.tile([65, TQ], f32, tag="dstg")
                        nc.vector.tensor_copy(dstg[64:65, :], op[64:65, :])
                        nc.sync.dma_start(
                            den_all[i % 64 : i % 64 + 1, i // 64, :],
                            dstg[64:65, :],
                        )

            # ================= Phase 5: normalize =================
            nc.vector.reciprocal(rden_all[:], den_all[:])
            nc.sync.dma_start(rden_dram[:], rden_all[:])
            with tc.tile_pool(name="bc_sb", bufs=6) as bc_sb:
                for h in range(HPC):
                    for qb in range(NQB):
                        q0 = qb * TQ
                        i = h * NQB + qb
                        bc = bc_sb.tile([64, TQ], f32, tag="bc")
                        nc.sync.dma_start(
                            bc[:],
                            rden_dram[i % 64 : i % 64 + 1, i // 64, :].broadcast_to(
                                (64, TQ)
                            ),
                        )
                        oslc = o_bf[:, h, q0 : q0 + TQ]
                        nc.vector.tensor_mul(oslc, oslc, bc[:])

            # ================= Phase 6: c_proj =================
            with tc.tile_pool(name="y_ps", bufs=4, space="PSUM") as y_ps, \
                 tc.tile_pool(name="y_sb", bufs=4) as y_sb:
                for tci in range(NTC):
                    ts = bass.ts(tci, 128)
                    for cc in range(2):
                        yp = y_ps.tile([128, 512], f32, tag="yp")
                        for h in range(HPC):
                            nc.tensor.matmul(
                                yp[:],
                                o_bf[:, h, ts],
                                cpw_sb[:, h, cc * 512 : (cc + 1) * 512],
                                start=(h == 0),
                                stop=(h == HPC - 1),
                            )
                        yt = y_sb.tile([128, 512], f32, tag="yt")
                        nc.scalar.copy(yt[:], yp[:])
                        nc.sync.dma_start(
                            y_d[ts, cc * 512 : (cc + 1) * 512], yt[:]
                        )

    n = _split_excess_waits(nc)
    print(f"[kernel] split {n} excess waits")
    return nc


def _rotary_tables():
    steps = HD // 4
    inv_freq = (1.0 / ROPE_BASE) ** np.linspace(0.0, 1.0, steps, dtype=np.float32)
    theta = np.arange(T, dtype=np.float32)[:, None] * inv_freq[None, :]  # [T,16]
    return np.cos(theta), np.sin(theta)


def kernel(x, v1, qkv_weight, c_proj_w, c_proj_b, lamb, attn_window):
    x = np.asarray(x, dtype=np.float32)
    v1 = np.asarray(v1, dtype=np.float32)
    qkv_weight = np.asarray(qkv_weight, dtype=np.float32)
    c_proj_w = np.asarray(c_proj_w, dtype=np.float32)
    c_proj_b = np.asarray(c_proj_b, dtype=np.float32)
    lam = float(np.asarray(lamb).reshape(-1)[0])
    assert int(attn_window) == W

    if "nc" not in _CACHE:
        _CACHE["nc"] = build_nc()
    nc = _CACHE["nc"]

    cos_t, sin_t = _rotary_tables()  # [T, 16]
    cos_rep = np.tile(cos_t, (1, HPC)).astype(ml_dtypes.bfloat16)  # [T, 128]
    sin_rep = np.tile(sin_t, (1, HPC)).astype(ml_dtypes.bfloat16)
    iden = np.eye(128, dtype=np.float32).astype(ml_dtypes.bfloat16)
    masks = np.stack([_mask_tile(r) for r in _mask_r_needed()]).astype(
        ml_dtypes.bfloat16
    )
    lam1m = np.full((128, 1), 1.0 - lam, dtype=np.float32)

    in_maps = []
    for core in range(8):
        b, hg = core // 2, core % 2
        dsl = slice(hg * D, (hg + 1) * D)
        in_maps.append(
            {
                "xT": np.ascontiguousarray(x[b].T).astype(ml_dtypes.bfloat16),
                "wq_t": np.ascontiguousarray(qkv_weight[0][dsl].T).astype(
                    ml_dtypes.bfloat16
                ),
                "wk_t": np.ascontiguousarray(qkv_weight[1][dsl].T).astype(
                    ml_dtypes.bfloat16
                ),
                "wv_t": np.ascontiguousarray(qkv_weight[2][dsl].T).astype(
                    ml_dtypes.bfloat16
                ),
                "cpw_t": np.ascontiguousarray(c_proj_w[:, dsl].T).astype(
                    ml_dtypes.bfloat16
                ),
                "v1s": (lam * v1[b][:, dsl]).astype(np.float32),
                "cos_r": cos_rep,
                "sin_r": sin_rep,
                "lam1m": lam1m,
                "ident_bf": iden,
                "masks": masks,
            }
        )

    import os
    trace = os.environ.get("KERNEL_TRACE", "0") == "1"
    res = run_bass_kernel_spmd(nc, in_maps, list(range(8)), trace=trace)
    _CACHE["last_result"] = res
    _CACHE["last_in_maps"] = in_maps

    y = np.zeros((B, T, C), dtype=np.float32)
    value = np.zeros((B, H, T, HD), dtype=np.float32)
    for core in range(8):
        b, hg = core // 2, core % 2
        out = res.results[core]
        y[b] += out["y_part"]
        vo = out["value_out"].reshape(T, HPC, HD)  # [T, 8, 64]
        value[b, hg * HPC : (hg + 1) * HPC] = vo.transpose(1, 0, 2)
    y += c_proj_b[None, None, :]
    return y, value


# revision 18
# speedup vs baseline: 1.1465x; 1.1465x over previous
"""Trainium2 Bass kernel for nn_CausalSelfAttention_55207509622867.

Sharding: 8 cores = 4 batches x 2 head-groups (8 heads each).
Per-core SPMD program (same IR, different data):
  - QKV projection (bf16 matmuls, fp32 PSUM accumulate)
  - RMS-norm + rotary on Q,K in natural [t,d] layout (DVE)
  - PE-transpose Q,K to [d,t] layout for attention
  - windowed-causal attention in S^T=[k,q] layout; mask via identity-matmul
    bias injection into PSUM (-400 out-of-band); exp on ScalarE;
    softmax denominator via an all-ones column appended to V (row 64 of O^T)
  - normalize O^T with PE-broadcast of 1/den, then c_proj matmuls
Host: pre-transposes/casts weights and x, blends value output, sums the
two head-group partial y's, adds bias, reassembles full outputs.
"""

import numpy as np
import ml_dtypes

import bass_rust
import concourse.bass as bass
import concourse.tile as tile
import concourse.mybir as mybir
from concourse.bass_utils import run_bass_kernel_spmd
from concourse.vector_clock import ScopedClock

# ----------------------------------------------------------------------------
# Tile exit-drain patch: this walrus build rejects sem waits on InstDrain
# ("Too many sync wait commands"). Put the global-clock waits on preceding
# sync-engine nops (1 wait each), then emit a bare drain.
# ----------------------------------------------------------------------------
def _patched_drain_and_barrier(self, tick_clock, wait_clock):
    nc = self.nc
    probe = nc.sync.nop(nofuse=True)
    wait_clock.add_sem_waits(probe.ins, ScopedClock({None: tick_clock.global_clock}))
    si = probe.ins.sync_info
    waits = [] if si is None else list(si.on_wait)
    if len(waits) > 1:
        probe.ins.sync_info = bass_rust.SyncInfo(
            on_wait=waits[:1], on_update=[] if si is None else list(si.on_update)
        )
        for w in waits[1:]:
            nop = nc.sync.nop(nofuse=True)
            nop.ins.sync_info = bass_rust.SyncInfo(on_wait=[w], on_update=[])
    nc.sync.drain()
    nc.all_engine_barrier()
    assert self.sems is not None
    popped = nc._tile_sem_poison_stack.pop()
    assert popped is self._sem_poison
    nc.clear_and_free_semaphores(list(self.sems.allocated().values()))
    nc.all_engine_barrier()


tile.TileContext._drain_and_barrier = _patched_drain_and_barrier

_MAXW = 1  # max sync waits this walrus accepts per instruction


def _split_excess_waits(nc):
    """Move waits beyond _MAXW onto same-engine nops inserted just before."""
    k = 0
    for bb in nc.m.functions[0].blocks:
        out = []
        for inst in bb.instructions:
            si = inst.sync_info
            waits = [] if si is None else list(si.on_wait)
            if len(waits) > _MAXW:
                extra = waits[:-_MAXW]
                inst.sync_info = bass_rust.SyncInfo(
                    on_wait=waits[-_MAXW:],
                    on_update=list(si.on_update),
                )
                for i in range(0, len(extra), _MAXW):
                    nop = mybir.InstNoOp(name=f"I-waitsplit-{k}", ins=[], outs=[])
                    k += 1
                    nop.engine = inst.engine
                    nop.sync_info = bass_rust.SyncInfo(
                        on_wait=extra[i : i + _MAXW], on_update=[]
                    )
                    out.append(nop)
            out.append(inst)
        if k:
            bb.instructions = out
    return k

# ----------------------------------------------------------------------------
# Problem constants (hardcoded per task contract)
# ----------------------------------------------------------------------------
B, T, C, H, HD = 4, 2048, 1024, 16, 64
W = 512                      # attention window
HPC = 8                      # heads per core
D = HPC * HD                 # 512 head-dims per core
TQ = 256                     # query block
NQB = T // TQ                # 8 query blocks
KT = 128                     # key tile
ROPE_BASE = 1024.0
EPS = 1e-6
NEG = -400.0                 # mask bias (exp(0.125*(-400+s)) == 0)

f32 = mybir.dt.float32
f32r = mybir.dt.float32r
bf16 = mybir.dt.bfloat16
FP = mybir.AluOpType
AF = mybir.ActivationFunctionType

_CACHE = {}


def _ktiles(qb):
    """Valid key-tile start indices (in units of 128) for query block qb."""
    q0 = qb * TQ
    out = []
    for j in range(6):
        k0 = q0 - W + j * KT
        if k0 < 0 or k0 >= T:
            continue
        out.append((j, k0))
    return out


def _mask_r_needed():
    """r = q0-k0 values whose [128,TQ] tile is partially masked."""
    return [-128, 0, 384, 512]


def _mask_tile(r):
    """[128, TQ] fp32 additive-bias tile for relative offset r."""
    p = np.arange(KT)[:, None]
    f = np.arange(TQ)[None, :]
    qk = r + f - p
    ok = (qk >= 0) & (qk < W)
    return np.where(ok, 0.0, NEG).astype(np.float32)


def build_nc(reps=1):
    nc = bass.Bass()

    # ---- DRAM parameters (per core) ----
    xT_d = nc.declare_dram_parameter("xT", [C, T], bf16, isOutput=False)
    wq_d = nc.declare_dram_parameter("wq_t", [C, D], bf16, isOutput=False)
    wk_d = nc.declare_dram_parameter("wk_t", [C, D], bf16, isOutput=False)
    wv_d = nc.declare_dram_parameter("wv_t", [C, D], bf16, isOutput=False)
    cpw_d = nc.declare_dram_parameter("cpw_t", [D, C], bf16, isOutput=False)
    v1s_d = nc.declare_dram_parameter("v1s", [T, D], f32, isOutput=False)
    cos_d = nc.declare_dram_parameter("cos_r", [T, HPC * 16], bf16, isOutput=False)
    sin_d = nc.declare_dram_parameter("sin_r", [T, HPC * 16], bf16, isOutput=False)
    lam_d = nc.declare_dram_parameter("lam1m", [128, 1], f32, isOutput=False)
    iden_d = nc.declare_dram_parameter("ident_bf", [128, 128], bf16, isOutput=False)
    mask_d = nc.declare_dram_parameter("masks", [4, 128, TQ], bf16, isOutput=False)

    y_d = nc.declare_dram_parameter("y_part", [T, C], f32, isOutput=True)
    val_d = nc.declare_dram_parameter("value_out", [T, D], f32, isOutput=True)
    rden_dram = nc.dram_tensor("rden_scratch", [64, NQB * HPC // 64, TQ], f32)

    NTC = T // 128  # 16 row tiles
    r_list = _mask_r_needed()
    r_slot = {r: i for i, r in enumerate(r_list)}

    with tile.TileContext(nc) as tc:
        from contextlib import ExitStack

        with ExitStack() as root:
            # ------------- persistent tensors -------------
            consts = root.enter_context(tc.tile_pool(name="consts", bufs=1))
            wq_sb = consts.tile([128, 8, D], bf16, name="wq_sb")
            wk_sb = consts.tile([128, 8, D], bf16, name="wk_sb")
            wv_sb = consts.tile([128, 8, D], bf16, name="wv_sb")
            cpw_sb = consts.tile([64, HPC, C], bf16, name="cpw_sb")
            cos_sb = consts.tile([128, NTC, HPC * 16], bf16, name="cos_sb")
            sin_sb = consts.tile([128, NTC, HPC * 16], bf16, name="sin_sb")
            lam_sb = consts.tile([128, 1], f32, name="lam_sb")
            iden_sb = consts.tile([128, 128], bf16, name="iden_sb")
            mask_sb = consts.tile([128, 4, TQ], bf16, name="mask_sb")

            for _ct in range(8):
                nc.sync.dma_start(wq_sb[:, _ct, :], wq_d[_ct * 128 : (_ct + 1) * 128, :])
            for _ct in range(8):
                nc.sync.dma_start(wk_sb[:, _ct, :], wk_d[_ct * 128 : (_ct + 1) * 128, :])
            for _ct in range(8):
                nc.sync.dma_start(wv_sb[:, _ct, :], wv_d[_ct * 128 : (_ct + 1) * 128, :])
            for _h in range(HPC):
                nc.sync.dma_start(cpw_sb[:, _h, :], cpw_d[_h * 64 : (_h + 1) * 64, :])
            nc.sync.dma_start(lam_sb[:], lam_d[:])
            nc.sync.dma_start(iden_sb[:], iden_d[:])
            # masks [4,128,TQ] -> sbuf [128, 4, TQ]
            for i in range(4):
                nc.sync.dma_start(mask_sb[:, i, :], mask_d[i, :, :])
            for tci in range(NTC):
                nc.sync.dma_start(
                    cos_sb[:, tci, :], cos_d[tci * 128 : (tci + 1) * 128, :]
                )
                nc.sync.dma_start(
                    sin_sb[:, tci, :], sin_d[tci * 128 : (tci + 1) * 128, :]
                )

            rep_cm = tc.For_i(0, reps, 1) if reps > 1 else None
            if rep_cm is not None:
                rep_cm.__enter__()
            main = root.enter_context(tc.tile_pool(name="main", bufs=1))
            # V augmented with ones col per head: [t, 8*65]
            v_aug = main.tile([128, NTC, HPC * 65], bf16, name="v_aug")
            # transposed Q,K: head-pair tensors [128, T] bf16
            qT = main.tile([128, 4, T], bf16, name="qT")
            kT = main.tile([128, 4, T], bf16, name="kT")
            # O (unnormalized then normalized in place), per-head, base partition 0
            o_bf = main.tile([64, HPC, T], bf16, name="o_bf")
            den_all = main.tile([64, NQB * HPC // 64, TQ], f32, name="den_all")
            rden_all = main.tile([64, NQB * HPC // 64, TQ], f32, name="rden_all")
            eps_sb = main.tile([128, 1], f32, name="eps_sb")
            nc.gpsimd.memset(eps_sb[:], EPS)

            # ========== Phase 1-3: QKV + rms/rotary + transpose (per tci) ==========
            with tc.tile_pool(name="xload", bufs=8) as xpool, \
                 tc.tile_pool(name="v1load", bufs=3) as v1pool, \
                 tc.tile_pool(name="qkv_ps", bufs=3, space="PSUM") as qkv_ps, \
                 tc.tile_pool(name="valout", bufs=3) as valpool, \
                 tc.tile_pool(name="qknat", bufs=3) as qkpool, \
                 tc.tile_pool(name="ssq", bufs=2) as ssqpool, \
                 tc.tile_pool(name="rstdp", bufs=2) as rstdpool, \
                 tc.tile_pool(name="rot", bufs=4) as rot, \
                 tc.tile_pool(name="tps", bufs=2, space="PSUM") as tps:

                xT_sb = [None] * 8
                for ct in range(8):
                    xt = xpool.tile([128, T], bf16, name=f"xt{ct}", tag="xt")
                    nc.sync.dma_start(xt[:], xT_d[ct * 128 : (ct + 1) * 128, :])
                    xT_sb[ct] = xt

                for tci in range(NTC):
                    ts = bass.ts(tci, 128)
                    qk_nat = qkpool.tile([128, 2 * D], bf16, tag="qkn")
                    # --- Q and K ---
                    for qk, w_sb in ((0, wq_sb), (1, wk_sb)):
                        ps = qkv_ps.tile([128, D], f32, tag="ps")
                        for ct in range(8):
                            nc.tensor.matmul(
                                ps[:],
                                xT_sb[ct][:, ts],
                                w_sb[:, ct, :],
                                start=(ct == 0),
                                stop=(ct == 7),
                            )
                        nc.scalar.copy(qk_nat[:, qk * D : (qk + 1) * D], ps[:])
                    # --- V + blend ---
                    ps = qkv_ps.tile([128, D], f32, tag="ps")
                    for ct in range(8):
                        nc.tensor.matmul(
                            ps[:],
                            xT_sb[ct][:, ts],
                            wv_sb[:, ct, :],
                            start=(ct == 0),
                            stop=(ct == 7),
                        )
                    v1t = v1pool.tile([128, D], f32, tag="v1")
                    nc.sync.dma_start(v1t[:], v1s_d[ts, :])
                    # value = (1-lam)*V + v1s   (fp32, DMA out)
                    vout = valpool.tile([128, D], f32, tag="vo")
                    nc.vector.scalar_tensor_tensor(
                        vout[:], ps[:], lam_sb[:], v1t[:], op0=FP.mult, op1=FP.add
                    )
                    nc.sync.dma_start(val_d[ts, :], vout[:])
                    # V_aug (bf16, gapped by head with ones col)
                    va = v_aug[:, tci, :].rearrange("p (h e) -> p h e", h=HPC)
                    nc.vector.scalar_tensor_tensor(
                        va[:, :, 0:HD],
                        ps[:].rearrange("p (h e) -> p h e", h=HPC),
                        lam_sb[:],
                        v1t[:].rearrange("p (h e) -> p h e", h=HPC),
                        op0=FP.mult,
                        op1=FP.add,
                    )
                    nc.gpsimd.memset(va[:, :, HD : HD + 1], 1.0)

                    # --- rms stats ---
                    sq = ssqpool.tile([128, 2 * D], bf16, tag="sq")
                    nc.vector.tensor_mul(sq[:], qk_nat[:], qk_nat[:])
                    sumsq = rstdpool.tile([128, 2, HPC], f32, tag="ssq")
                    nc.vector.tensor_reduce(
                        sumsq[:],
                        sq[:].rearrange("p (s h d) -> p (s h) d", s=2, h=HPC),
                        axis=mybir.AxisListType.X,
                        op=FP.add,
                    )
                    std_t = rstdpool.tile([128, 2, HPC], f32, tag="std")
                    nc.scalar.activation(
                        std_t[:], sumsq[:], AF.Sqrt, bias=eps_sb[:], scale=1.0 / HD
                    )
                    rstd = rstdpool.tile([128, 2, HPC], f32, tag="rstd")
                    nc.vector.reciprocal(rstd[:], std_t[:])

                    # --- apply rstd + rotary (in place on qk_nat) ---
                    nat = qk_nat[:].rearrange("p (s h d) -> p s h d", s=2, h=HPC)
                    nc.vector.tensor_mul(
                        nat[:],
                        nat[:],
                        rstd[:][:, :, :, None].broadcast_to((128, 2, HPC, HD)),
                    )
                    x1 = nat[:, :, :, 0:16]
                    x2 = nat[:, :, :, 32:48]
                    csb = cos_sb[:, tci, :].rearrange("p (h j) -> p h j", h=HPC)[
                        :, None, :, :
                    ].broadcast_to((128, 2, HPC, 16))
                    ssb = sin_sb[:, tci, :].rearrange("p (h j) -> p h j", h=HPC)[
                        :, None, :, :
                    ].broadcast_to((128, 2, HPC, 16))
                    tb = rot.tile([128, 2, HPC, 16], bf16, tag="tb")
                    td = rot.tile([128, 2, HPC, 16], bf16, tag="td")
                    nc.vector.tensor_mul(tb[:], x1, ssb)   # x1*sin
                    nc.vector.tensor_mul(td[:], x2, ssb)   # x2*sin
                    nc.vector.tensor_mul(x1, x1, csb)      # x1*cos
                    nc.vector.tensor_add(x1, x1, td[:])    # + x2*sin
                    nc.vector.tensor_mul(x2, x2, csb)      # x2*cos
                    nc.vector.tensor_sub(x2, x2, tb[:])    # - x1*sin

                    # --- transpose to [d, t] ---
                    for qk, dst in ((0, qT), (1, kT)):
                        tp = tps.tile([128, 512], bf16, tag="tp")
                        for pair in range(4):
                            nc.tensor.transpose(
                                tp[:, pair * 128 : (pair + 1) * 128],
                                qk_nat[
                                    :, qk * D + pair * 128 : qk * D + (pair + 1) * 128
                                ],
                                iden_sb[:],
                            )
                        nc.vector.tensor_copy(
                            dst[:, :, ts],
                            tp[:].rearrange("p (r c) -> p r c", r=4),
                        )

            # ================= Phase 4: attention =================
            with tc.tile_pool(name="s_ps", bufs=2, space="PSUM") as s_ps, \
                 tc.tile_pool(name="o_ps", bufs=2, space="PSUM") as o_ps, \
                 tc.tile_pool(name="pt_sb", bufs=2) as pt_pool, \
                 tc.tile_pool(name="den_stage", bufs=3) as den_stage:

                for h in range(HPC):
                    pair, half = h // 2, h % 2
                    hslc = slice(half * 64, half * 64 + 64)
                    for qb in range(NQB):
                        q0 = qb * TQ
                        kts = _ktiles(qb)
                        j0 = kts[0][0]
                        sp = s_ps.tile([128, 6 * TQ], f32, tag="sp")
                        for j, k0 in kts:
                            r = q0 - k0
                            slc = slice(j * TQ, (j + 1) * TQ)
                            masked = r in r_slot
                            if masked:
                                nc.tensor.matmul(
                                    sp[:, slc],
                                    iden_sb[:],
                                    mask_sb[:, r_slot[r], :],
                                    start=True,
                                    stop=False,
                                )
                            nc.tensor.matmul(
                                sp[:, slc],
                                kT[hslc, pair, k0 : k0 + 128],
                                qT[hslc, pair, q0 : q0 + TQ],
                                start=not masked,
                                stop=True,
                            )
                        # exp over the valid tail
                        pt = pt_pool.tile([128, 6 * TQ], bf16, tag="pt")
                        nc.scalar.activation(
                            pt[:, j0 * TQ :], sp[:, j0 * TQ :], AF.Exp,
                            scale=1.0 / np.sqrt(HD),
                        )
                        # AV: O^T[65, TQ] accumulate over key tiles
                        op = o_ps.tile([65, TQ], f32, tag="op")
                        for idx, (j, k0) in enumerate(kts):
                            nc.tensor.matmul(
                                op[:],
                                v_aug[:, k0 // 128, h * 65 : (h + 1) * 65],
                                pt[:, j * TQ : (j + 1) * TQ],
                                start=(idx == 0),
                                stop=(idx == len(kts) - 1),
                            )
                        nc.vector.tensor_copy(
                            o_bf[:, h, q0 : q0 + TQ], op[0:64, :]
                        )
                        i = h * NQB + qb
                        dstg = den_stage.tile([65, TQ], f32, tag="dstg")
                        nc.vector.tensor_copy(dstg[64:65, :], op[64:65, :])
                        nc.sync.dma_start(
                            den_all[i % 64 : i % 64 + 1, i // 64, :],
                            dstg[64:65, :],
                        )

            # ================= Phase 5: normalize =================
            nc.vector.reciprocal(rden_all[:], den_all[:])
            nc.sync.dma_start(rden_dram[:], rden_all[:])
            with tc.tile_pool(name="bc_sb", bufs=6) as bc_sb:
                for h in range(HPC):
                    for qb in range(NQB):
                        q0 = qb * TQ
                        i = h * NQB + qb
                        bc = bc_sb.tile([64, TQ], f32, tag="bc")
                        nc.sync.dma_start(
                            bc[:],
                            rden_dram[i % 64 : i % 64 + 1, i // 64, :].broadcast_to(
                                (64, TQ)
                            ),
                        )
                        oslc = o_bf[:, h, q0 : q0 + TQ]
                        nc.vector.tensor_mul(oslc, oslc, bc[:])

            # ================= Phase 6: c_proj =================
            with tc.tile_pool(name="y_ps", bufs=4, space="PSUM") as y_ps, \
                 tc.tile_pool(name="y_sb", bufs=4) as y_sb:
                for tci in range(NTC):
                    ts = bass.ts(tci, 128)
                    for cc in range(2):
                        yp = y_ps.tile([128, 512], f32, tag="yp")
                        for h in range(HPC):
                            nc.tensor.matmul(
                                yp[:],
                                o_bf[:, h, ts],
                                cpw_sb[:, h, cc * 512 : (cc + 1) * 512],
                                start=(h == 0),
                                stop=(h == HPC - 1),
                            )
                        yt = y_sb.tile([128, 512], f32, tag="yt")
                        nc.scalar.copy(yt[:], yp[:])
                        nc.sync.dma_start(
                            y_d[ts, cc * 512 : (cc + 1) * 512], yt[:]
                        )

            if rep_cm is not None:
                rep_cm.__exit__(None, None, None)
    n = _split_excess_waits(nc)
    print(f"[kernel] split {n} excess waits")
    return nc


def _rotary_tables():
    steps = HD // 4
    inv_freq = (1.0 / ROPE_BASE) ** np.linspace(0.0, 1.0, steps, dtype=np.float32)
    theta = np.arange(T, dtype=np.float32)[:, None] * inv_freq[None, :]  # [T,16]
    return np.cos(theta), np.sin(theta)


def kernel(x, v1, qkv_weight, c_proj_w, c_proj_b, lamb, attn_window):
    x = np.asarray(x, dtype=np.float32)
    v1 = np.asarray(v1, dtype=np.float32)
    qkv_weight = np.asarray(qkv_weight, dtype=np.float32)
    c_proj_w = np.asarray(c_proj_w, dtype=np.float32)
    c_proj_b = np.asarray(c_proj_b, dtype=np.float32)
    lam = float(np.asarray(lamb).reshape(-1)[0])
    assert int(attn_window) == W

    if "nc" not in _CACHE:
        _CACHE["nc"] = build_nc()
    nc = _CACHE["nc"]

    cos_t, sin_t = _rotary_tables()  # [T, 16]
    cos_rep = np.tile(cos_t, (1, HPC)).astype(ml_dtypes.bfloat16)  # [T, 128]
    sin_rep = np.tile(sin_t, (1, HPC)).astype(ml_dtypes.bfloat16)
    iden = np.eye(128, dtype=np.float32).astype(ml_dtypes.bfloat16)
    masks = np.stack([_mask_tile(r) for r in _mask_r_needed()]).astype(
        ml_dtypes.bfloat16
    )
    lam1m = np.full((128, 1), 1.0 - lam, dtype=np.float32)

    in_maps = []
    for core in range(8):
        b, hg = core // 2, core % 2
        dsl = slice(hg * D, (hg + 1) * D)
        in_maps.append(
            {
                "xT": np.ascontiguousarray(x[b].T).astype(ml_dtypes.bfloat16),
                "wq_t": np.ascontiguousarray(qkv_weight[0][dsl].T).astype(
                    ml_dtypes.bfloat16
                ),
                "wk_t": np.ascontiguousarray(qkv_weight[1][dsl].T).astype(
                    ml_dtypes.bfloat16
                ),
                "wv_t": np.ascontiguousarray(qkv_weight[2][dsl].T).astype(
                    ml_dtypes.bfloat16
                ),
                "cpw_t": np.ascontiguousarray(c_proj_w[:, dsl].T).astype(
                    ml_dtypes.bfloat16
                ),
                "v1s": (lam * v1[b][:, dsl]).astype(np.float32),
                "cos_r": cos_rep,
                "sin_r": sin_rep,
                "lam1m": lam1m,
                "ident_bf": iden,
                "masks": masks,
            }
        )

    import os
    trace = os.environ.get("KERNEL_TRACE", "0") == "1"
    res = run_bass_kernel_spmd(nc, in_maps, list(range(8)), trace=trace)
    _CACHE["last_result"] = res
    _CACHE["last_in_maps"] = in_maps

    y = np.zeros((B, T, C), dtype=np.float32)
    value = np.zeros((B, H, T, HD), dtype=np.float32)
    for core in range(8):
        b, hg = core // 2, core % 2
        out = res.results[core]
        y[b] += out["y_part"]
        vo = out["value_out"].reshape(T, HPC, HD)  # [T, 8, 64]
        value[b, hg * HPC : (hg + 1) * HPC] = vo.transpose(1, 0, 2)
    y += c_proj_b[None, None, :]
    return y, value
